# revision 3
# baseline (speedup 1.0000x reference)
"""Trainium2 Bass kernel for nn_CSCFCLayer: out = relu(x @ W + b).

Shapes: x [4096, 4096] f32, W [4096, 4096] f32, b [4096] f32 -> out [4096, 4096] f32.

Sharding: batch (rows of x) split 8 ways -- each core computes a [512, 4096]
slice of the output.  x is pre-transposed on the host so the contraction dim
lands on SBUF partitions with no on-device transposes; the output is produced
transposed (units on partitions) so the bias is a per-partition scalar and
ScalarE fuses bias+relu in a single ACTIVATE reading PSUM.

Per core:
  xT_c [K=4096, M=512]  (resident in SBUF, 8 MiB)
  W    [K=4096, N=4096] (streamed, 64 MiB)
  outT_c [N=4096, M=512]
  loop g in 8 groups of 4 unit-chunks (128 units each):
    loop k in 32:  DMA W k-tile [128, 512]; 4 matmuls (lhsT=W chunk, rhs=xT)
    4x ACT relu(psum + bias) -> SBUF -> DMA out
"""

import os

import numpy as np

import concourse.bass as bass
import concourse.tile as tile
from concourse import bacc, mybir
from concourse.bass_utils import run_bass_kernel_spmd

N_CORES = 8
B = 4096
K = 4096
N = 4096
BS = B // N_CORES  # 512 batch rows per core
P = 128
KT = K // P  # 32 k-tiles
NCHUNK = N // P  # 32 unit-chunks of 128
NG = N // 512  # 8 groups of 4 unit-chunks

# matmul input dtype: float32r = full-rate fp32 path on the PE
MM_DT = {
    "f32r": mybir.dt.float32r,
    "f32": mybir.dt.float32,
    "bf16": mybir.dt.bfloat16,
}[os.environ.get("CSCFC_MM_DT", "f32r")]


def _body(tc, xt, w, bt, outT):
    nc = tc.nc
    with (
        tc.tile_pool(name="xpool", bufs=1) as xpool,
        tc.tile_pool(name="wpool", bufs=8) as wpool,
        tc.tile_pool(name="bpool", bufs=1) as bpool,
        tc.tile_pool(name="opool", bufs=8) as opool,
        tc.tile_pool(name="psum", bufs=8, space="PSUM") as psum_pool,
    ):
        # Resident x.T: [128 (k-partition), 32 (k-tile), 512 (batch)]
        xt_r = xt.rearrange("(kt p) m -> p kt m", p=P)
        xt_sb = xpool.tile([P, KT, BS], xt.dtype)
        n_pre = min(8, KT)  # split preload so the first matmuls start early
        for c in range(n_pre):
            sl = slice(c * (KT // n_pre), (c + 1) * (KT // n_pre))
            nc.sync.dma_start(xt_sb[:, sl, :], xt_r[:, sl, :])

        bias_sb = bpool.tile([P, NCHUNK], mybir.dt.float32)
        nc.sync.dma_start(bias_sb[:], bt[:, :])

        for g in range(NG):
            psums = [
                psum_pool.tile([P, BS], mybir.dt.float32, tag="ps", name=f"ps_{g}_{j}")
                for j in range(4)
            ]
            for k in range(KT):
                wt = wpool.tile([P, 512], w.dtype, tag="wt", name=f"wt_{g}_{k}")
                nc.sync.dma_start(wt[:], w[k * P : (k + 1) * P, g * 512 : (g + 1) * 512])
                for j in range(4):
                    nc.tensor.matmul(
                        psums[j][:],
                        wt[:, j * P : (j + 1) * P],
                        xt_sb[:, k, :],
                        start=(k == 0),
                        stop=(k == KT - 1),
                    )
            for j in range(4):
                ch = g * 4 + j
                ot = opool.tile([P, BS], mybir.dt.float32, tag="ot", name=f"ot_{ch}")
                nc.scalar.activation(
                    ot[:],
                    psums[j][:],
                    mybir.ActivationFunctionType.Relu,
                    bias=bias_sb[:, ch : ch + 1],
                )
                nc.sync.dma_start(outT[ch * P : (ch + 1) * P, :], ot[:])


def build_nc(mm_dt=MM_DT):
    nc = bacc.Bacc("TRN2", target_bir_lowering=False, debug=False)
    xt = nc.dram_tensor("xt", (K, BS), mm_dt, kind="ExternalInput")
    w = nc.dram_tensor("w", (K, N), mm_dt, kind="ExternalInput")
    bt = nc.dram_tensor("bt", (P, NCHUNK), mybir.dt.float32, kind="ExternalInput")
    outT = nc.dram_tensor("outT", (N, BS), mybir.dt.float32, kind="ExternalOutput")
    with tile.TileContext(nc) as tc:
        _body(tc, xt.ap(), w.ap(), bt.ap(), outT.ap())
    nc.compile()
    return nc


_CACHED_NC = None


def _get_nc():
    global _CACHED_NC
    if _CACHED_NC is None:
        _CACHED_NC = build_nc()
    return _CACHED_NC


def _run(nc, x, w, bias, **spmd_kwargs):
    np_dt = mybir.dt.np(MM_DT)
    xT = np.ascontiguousarray(x.T.astype(np_dt, copy=False))  # [K, B]
    w_full = np.ascontiguousarray(w.astype(np_dt, copy=False))
    bt = np.ascontiguousarray(bias.astype(np.float32).reshape(NCHUNK, P).T)
    in_maps = [
        {
            "xt": np.ascontiguousarray(xT[:, c * BS : (c + 1) * BS]),
            "w": w_full,
            "bt": bt,
        }
        for c in range(N_CORES)
    ]
    res = run_bass_kernel_spmd(nc, in_maps, list(range(N_CORES)), **spmd_kwargs)
    out = np.empty((B, N), dtype=np.float32)
    for c in range(N_CORES):
        out[c * BS : (c + 1) * BS, :] = res.results[c]["outT"].T
    return out, res


def kernel(x, kernel, bias):
    out, _ = _run(_get_nc(), x, kernel, bias)
    return out


# revision 15
# speedup vs baseline: 1.0086x; 1.0086x over previous
"""Trainium2 Bass kernel for nn_CSCFCLayer: out = relu(x @ W + b).

Shapes: x [4096, 4096] f32, W [4096, 4096] f32, b [4096] f32 -> out [4096, 4096] f32.

Sharding: 2D over 8 cores -- batch split 4 ways x units split 2 ways. Each core
computes a [1024, 2048] slice of the output. Per-core DMA: xT slice 16 MiB +
W slice 32 MiB + out 8 MiB = 56 MiB (vs 80 MiB for 1D sharding), which puts the
kernel in the compute-bound regime (PE floor ~209 us at the measured 204 ns per
fp32r matmul).

x is pre-transposed on the host so the contraction dim lands on SBUF partitions
with no on-device transposes. The output is produced transposed (units on
partitions) so bias is a per-partition scalar and ScalarE fuses bias+relu in a
single ACTIVATE reading PSUM. Matmuls run in float32r (full-rate fp32 PE path).

Per core:
  xT_c [K=4096, M=1024] resident in SBUF (16 MiB)
  W_c  [K=4096, N=2048] streamed once (32 MiB)
  outT_c [N=2048, M=1024]
  for ng in 4 groups of 4 unit-chunks (512 units per group):
    for k in 32:  DMA W k-tile [128, 512]; 8 matmuls (4 chunks x 2 m-halves)
    8x ACT relu(psum + bias) -> SBUF -> DMA out
"""

import os

import numpy as np

import concourse.bass as bass
import concourse.tile as tile
from concourse import bacc, mybir
from concourse.bass_utils import run_bass_kernel_spmd

N_CORES = 8
P_SHARD = 4  # batch split
Q_SHARD = 2  # units split
B = 4096
K = 4096
N = 4096
BS = B // P_SHARD  # 1024 batch rows per core
NS = N // Q_SHARD  # 2048 units per core
P = 128
KT = K // P  # 32 k-tiles
NCHUNK = NS // P  # 16 unit-chunks of 128 per core
NG = NS // 512  # 4 groups of 4 unit-chunks
MH = BS // 512  # 2 moving halves of the batch

MM_DT = {
    "f32r": mybir.dt.float32r,
    "f32": mybir.dt.float32,
    "bf16": mybir.dt.bfloat16,
}[os.environ.get("CSCFC_MM_DT", "f32r")]


def _emit_loads(nc, xpool, bpool, xt, bt):
    # Resident x.T: [128 (k-partition), 32 (k-tile), 1024 (batch)]
    xt_r = xt.rearrange("(kt p) m -> p kt m", p=P)
    xt_sb = xpool.tile([P, KT, BS], xt.dtype, tag="xt_sb", name="xt_sb")
    n_pre = min(16, KT)  # split preload so the first matmuls start early
    for c in range(n_pre):
        sl = slice(c * (KT // n_pre), (c + 1) * (KT // n_pre))
        nc.sync.dma_start(xt_sb[:, sl, :], xt_r[:, sl, :])

    bias_sb = bpool.tile([P, NCHUNK], mybir.dt.float32, tag="bias_sb", name="bias_sb")
    nc.sync.dma_start(bias_sb[:], bt[:, :])
    return xt_sb, bias_sb


def _emit(nc, wpool, opool, psum_pool, xt_sb, bias_sb, w, outT):
    # groups of CPG unit-chunks x MH m-halves of PSUM banks.  j-innermost so
    # the MOVING operand (xt slice) stays fixed for CPG consecutive matmuls --
    # changing the moving AP per-mm costs ~70 ns/mm on HW.  The epilogue is a
    # single DVE tensor_scalar (psum + bias per-partition, then max 0) -- the
    # ScalarE ACTIVATE drain is ~2.5 us and serializes against PE PSUM writes.
    CPG = 4  # unit-chunks per group
    KTW = min(8, KT)  # k-tiles per W fetch block
    w_r = w.rearrange("(kb p) n -> p kb n", p=P)
    for g in range(NCHUNK // CPG):
        psums = [
            psum_pool.tile([P, 512], mybir.dt.float32, tag="ps", name=f"ps_{g}_{u}")
            for u in range(CPG * MH)
        ]
        for kb in range(KT // KTW):
            wt = wpool.tile([P, KTW, CPG * P], w.dtype, tag="wt", name=f"wt_{g}_{kb}")
            nc.sync.dma_start(
                wt[:],
                w_r[:, kb * KTW : (kb + 1) * KTW, g * CPG * P : (g + 1) * CPG * P],
            )
            for ki in range(KTW):
                k = kb * KTW + ki
                for mh in range(MH):
                    for j in range(CPG):
                        nc.tensor.matmul(
                            psums[mh * CPG + j][:],
                            wt[:, ki, j * P : (j + 1) * P],
                            xt_sb[:, k, mh * 512 : (mh + 1) * 512],
                            start=(k == 0),
                            stop=(k == KT - 1),
                        )
        for mh in range(MH):
            for j in range(CPG):
                ch = g * CPG + j
                ot = opool.tile(
                    [P, 512], mybir.dt.float32, tag="ot", name=f"ot_{ch}_{mh}"
                )
                nc.vector.tensor_scalar(
                    ot[:],
                    psums[mh * CPG + j][:],
                    bias_sb[:, ch : ch + 1],
                    0.0,
                    mybir.AluOpType.add,
                    mybir.AluOpType.max,
                )
                nc.sync.dma_start(
                    outT[ch * P : (ch + 1) * P, mh * 512 : (mh + 1) * 512], ot[:]
                )


def _body(tc, xt, w, bt, outT, reps=1):
    nc = tc.nc
    with (
        tc.tile_pool(name="xpool", bufs=1) as xpool,
        tc.tile_pool(name="wpool", bufs=2) as wpool,
        tc.tile_pool(name="bpool", bufs=1) as bpool,
        tc.tile_pool(name="opool", bufs=8) as opool,
        tc.tile_pool(name="psum", bufs=8, space="PSUM") as psum_pool,
    ):
        xt_sb, bias_sb = _emit_loads(nc, xpool, bpool, xt, bt)
        if reps > 1:
            # steady-state timing mode: resident loads stay outside the loop
            with tc.For_i(0, reps, 1):
                _emit(nc, wpool, opool, psum_pool, xt_sb, bias_sb, w, outT)
        else:
            _emit(nc, wpool, opool, psum_pool, xt_sb, bias_sb, w, outT)


def build_nc(mm_dt=MM_DT, reps=1):
    nc = bacc.Bacc("TRN2", target_bir_lowering=False, debug=False)
    xt = nc.dram_tensor("xt", (K, BS), mm_dt, kind="ExternalInput")
    w = nc.dram_tensor("w", (K, NS), mm_dt, kind="ExternalInput")
    bt = nc.dram_tensor("bt", (P, NCHUNK), mybir.dt.float32, kind="ExternalInput")
    outT = nc.dram_tensor("outT", (NS, BS), mybir.dt.float32, kind="ExternalOutput")
    with tile.TileContext(nc) as tc:
        _body(tc, xt.ap(), w.ap(), bt.ap(), outT.ap(), reps=reps)
    nc.compile()
    return nc


_CACHED_NC = None


def _get_nc():
    global _CACHED_NC
    if _CACHED_NC is None:
        _CACHED_NC = build_nc()
    return _CACHED_NC


def make_in_maps(x, w, bias):
    np_dt = mybir.dt.np(MM_DT)
    xT = np.ascontiguousarray(x.T.astype(np_dt, copy=False))  # [K, B]
    wc = w.astype(np_dt, copy=False)
    bias = bias.astype(np.float32, copy=False)
    in_maps = []
    for c in range(N_CORES):
        pi, qi = divmod(c, Q_SHARD)
        in_maps.append(
            {
                "xt": np.ascontiguousarray(xT[:, pi * BS : (pi + 1) * BS]),
                "w": np.ascontiguousarray(wc[:, qi * NS : (qi + 1) * NS]),
                "bt": np.ascontiguousarray(
                    bias[qi * NS : (qi + 1) * NS].reshape(NCHUNK, P).T
                ),
            }
        )
    return in_maps


def gather_out(results):
    out = np.empty((B, N), dtype=np.float32)
    for c in range(N_CORES):
        pi, qi = divmod(c, Q_SHARD)
        out[pi * BS : (pi + 1) * BS, qi * NS : (qi + 1) * NS] = results[c]["outT"].T
    return out


def _run(nc, x, w, bias, **spmd_kwargs):
    in_maps = make_in_maps(x, w, bias)
    res = run_bass_kernel_spmd(nc, in_maps, list(range(N_CORES)), **spmd_kwargs)
    return gather_out(res.results), res


def kernel(x, kernel, bias):
    out, _ = _run(_get_nc(), x, kernel, bias)
    return out


# revision 19
# speedup vs baseline: 10.7732x; 10.6808x over previous
"""Trainium2 Bass kernel for nn_CSCFCLayer: out = relu(x @ W + b).

Shapes: x [4096, 4096] f32, W [4096, 4096] f32, b [4096] f32 -> out [4096, 4096] f32.

Sharding: 2D over 8 cores -- batch split 4 ways x units split 2 ways. Each core
computes a [1024, 2048] slice of the output. Per-core DMA: xT slice 16 MiB +
W slice 32 MiB + out 8 MiB = 56 MiB (vs 80 MiB for 1D sharding), which puts the
kernel in the compute-bound regime (PE floor ~209 us at the measured 204 ns per
fp32r matmul).

x is pre-transposed on the host so the contraction dim lands on SBUF partitions
with no on-device transposes. The output is produced transposed (units on
partitions) so bias is a per-partition scalar and a single DVE tensor_scalar
(psum + bias, then max 0) fuses bias+relu while draining PSUM. Matmuls run in
float32r (full-rate fp32 PE path: 204 ns per [128x128x512] matmul measured).

Per core:
  xT_c [K=4096, M=1024] resident in SBUF (16 MiB)
  W_c  [K=4096, N=2048] streamed once (32 MiB) in [128, 8, 512] blocks
  outT_c [N=2048, M=1024]
  4 groups of (4 unit-chunks x 2 m-halves = 8 PSUM banks):
    k-accumulate 32 tiles; innermost over unit-chunks so the moving operand
    stays fixed for 4 consecutive matmuls; DVE bias+relu drain; DMA out.
"""

import os

import numpy as np

import concourse.bass as bass
import concourse.tile as tile
from concourse import bacc, mybir
from concourse.bass_utils import run_bass_kernel_spmd

N_CORES = 8
P_SHARD = 4  # batch split
Q_SHARD = 2  # units split
B = 4096
K = 4096
N = 4096
BS = B // P_SHARD  # 1024 batch rows per core
NS = N // Q_SHARD  # 2048 units per core
P = 128
KT = K // P  # 32 k-tiles
NCHUNK = NS // P  # 16 unit-chunks of 128 per core
NG = NS // 512  # 4 groups of 4 unit-chunks
MH = BS // 512  # 2 moving halves of the batch

MM_DT = {
    "f32r": mybir.dt.float32r,
    "f32": mybir.dt.float32,
    "bf16": mybir.dt.bfloat16,
}[os.environ.get("CSCFC_MM_DT", "f32r")]


def _emit_loads(nc, xpool, bpool, xt, bt):
    # Resident x.T: [128 (k-partition), 32 (k-tile), 1024 (batch)]
    xt_r = xt.rearrange("(kt p) m -> p kt m", p=P)
    xt_sb = xpool.tile([P, KT, BS], xt.dtype, tag="xt_sb", name="xt_sb")
    n_pre = min(16, KT)  # split preload so the first matmuls start early
    for c in range(n_pre):
        sl = slice(c * (KT // n_pre), (c + 1) * (KT // n_pre))
        nc.sync.dma_start(xt_sb[:, sl, :], xt_r[:, sl, :])

    bias_sb = bpool.tile([P, NCHUNK], mybir.dt.float32, tag="bias_sb", name="bias_sb")
    nc.sync.dma_start(bias_sb[:], bt[:, :])
    return xt_sb, bias_sb


def _emit(nc, wpool, opool, psum_pool, xt_sb, bias_sb, w, outT):
    # groups of CPG unit-chunks x MH m-halves of PSUM banks.  j-innermost so
    # the MOVING operand (xt slice) stays fixed for CPG consecutive matmuls --
    # changing the moving AP per-mm costs ~70 ns/mm on HW.  The epilogue is a
    # single DVE tensor_scalar (psum + bias per-partition, then max 0) -- the
    # ScalarE ACTIVATE drain is ~2.5 us and serializes against PE PSUM writes.
    CPG = 4  # unit-chunks per group
    KTW = min(8, KT)  # k-tiles per W fetch block
    w_r = w.rearrange("(kb p) n -> p kb n", p=P)
    for g in range(NCHUNK // CPG):
        psums = [
            psum_pool.tile([P, 512], mybir.dt.float32, tag="ps", name=f"ps_{g}_{u}")
            for u in range(CPG * MH)
        ]
        for kb in range(KT // KTW):
            wt = wpool.tile([P, KTW, CPG * P], w.dtype, tag="wt", name=f"wt_{g}_{kb}")
            nc.sync.dma_start(
                wt[:],
                w_r[:, kb * KTW : (kb + 1) * KTW, g * CPG * P : (g + 1) * CPG * P],
            )
            for ki in range(KTW):
                k = kb * KTW + ki
                for mh in range(MH):
                    for j in range(CPG):
                        nc.tensor.matmul(
                            psums[mh * CPG + j][:],
                            wt[:, ki, j * P : (j + 1) * P],
                            xt_sb[:, k, mh * 512 : (mh + 1) * 512],
                            start=(k == 0),
                            stop=(k == KT - 1),
                        )
        for mh in range(MH):
            for j in range(CPG):
                ch = g * CPG + j
                ot = opool.tile(
                    [P, 512], mybir.dt.float32, tag="ot", name=f"ot_{ch}_{mh}"
                )
                nc.vector.tensor_scalar(
                    ot[:],
                    psums[mh * CPG + j][:],
                    bias_sb[:, ch : ch + 1],
                    0.0,
                    mybir.AluOpType.add,
                    mybir.AluOpType.max,
                )
                nc.sync.dma_start(
                    outT[ch * P : (ch + 1) * P, mh * 512 : (mh + 1) * 512], ot[:]
                )


def _body(tc, xt, w, bt, outT, reps=1):
    nc = tc.nc
    with (
        tc.tile_pool(name="xpool", bufs=1) as xpool,
        tc.tile_pool(name="wpool", bufs=2) as wpool,
        tc.tile_pool(name="bpool", bufs=1) as bpool,
        tc.tile_pool(name="opool", bufs=8) as opool,
        tc.tile_pool(name="psum", bufs=8, space="PSUM") as psum_pool,
    ):
        xt_sb, bias_sb = _emit_loads(nc, xpool, bpool, xt, bt)
        if reps > 1:
            # steady-state timing mode: resident loads stay outside the loop
            with tc.For_i(0, reps, 1):
                _emit(nc, wpool, opool, psum_pool, xt_sb, bias_sb, w, outT)
        else:
            _emit(nc, wpool, opool, psum_pool, xt_sb, bias_sb, w, outT)


def build_nc(mm_dt=MM_DT, reps=1, full_reps=1):
    nc = bacc.Bacc("TRN2", target_bir_lowering=False, debug=False)
    xt = nc.dram_tensor("xt", (K, BS), mm_dt, kind="ExternalInput")
    w = nc.dram_tensor("w", (K, NS), mm_dt, kind="ExternalInput")
    bt = nc.dram_tensor("bt", (P, NCHUNK), mybir.dt.float32, kind="ExternalInput")
    outT = nc.dram_tensor("outT", (NS, BS), mybir.dt.float32, kind="ExternalOutput")
    with tile.TileContext(nc) as tc:
        if full_reps > 1:
            # timing variant: repeat the ENTIRE kernel (incl. resident loads)
            # so a reps-differential bounds the single-shot time from above
            nc2 = tc.nc
            with (
                tc.tile_pool(name="xpool", bufs=1) as xpool,
                tc.tile_pool(name="wpool", bufs=2) as wpool,
                tc.tile_pool(name="bpool", bufs=1) as bpool,
                tc.tile_pool(name="opool", bufs=8) as opool,
                tc.tile_pool(name="psum", bufs=8, space="PSUM") as psum_pool,
            ):
                with tc.For_i(0, full_reps, 1):
                    xt_sb, bias_sb = _emit_loads(nc2, xpool, bpool, xt.ap(), bt.ap())
                    _emit(nc2, wpool, opool, psum_pool, xt_sb, bias_sb, w.ap(), outT.ap())
        else:
            _body(tc, xt.ap(), w.ap(), bt.ap(), outT.ap(), reps=reps)
    nc.compile()
    return nc


_CACHED_NC = None


def _get_nc():
    global _CACHED_NC
    if _CACHED_NC is None:
        _CACHED_NC = build_nc()
    return _CACHED_NC


def make_in_maps(x, w, bias):
    x = np.asarray(x)
    w = np.asarray(w)
    bias = np.asarray(bias)
    np_dt = mybir.dt.np(MM_DT)
    xT = np.ascontiguousarray(x.T.astype(np_dt, copy=False))  # [K, B]
    wc = w.astype(np_dt, copy=False)
    bias = bias.astype(np.float32, copy=False)
    in_maps = []
    for c in range(N_CORES):
        pi, qi = divmod(c, Q_SHARD)
        in_maps.append(
            {
                "xt": np.ascontiguousarray(xT[:, pi * BS : (pi + 1) * BS]),
                "w": np.ascontiguousarray(wc[:, qi * NS : (qi + 1) * NS]),
                "bt": np.ascontiguousarray(
                    bias[qi * NS : (qi + 1) * NS].reshape(NCHUNK, P).T
                ),
            }
        )
    return in_maps


def gather_out(results):
    out = np.empty((B, N), dtype=np.float32)
    for c in range(N_CORES):
        pi, qi = divmod(c, Q_SHARD)
        out[pi * BS : (pi + 1) * BS, qi * NS : (qi + 1) * NS] = results[c]["outT"].T
    return out


def _run(nc, x, w, bias, **spmd_kwargs):
    in_maps = make_in_maps(x, w, bias)
    res = run_bass_kernel_spmd(nc, in_maps, list(range(N_CORES)), **spmd_kwargs)
    return gather_out(res.results), res


def kernel(x, kernel, bias):
    try:
        out, _ = _run(_get_nc(), x, kernel, bias)
    except Exception:
        # transient device errors (e.g. NRT_EXEC_UNIT_UNRECOVERABLE) recover
        # on re-execution
        out, _ = _run(_get_nc(), x, kernel, bias)
    return out


# revision 24
# speedup vs baseline: 11.2503x; 1.0443x over previous
"""Trainium2 Bass kernel for nn_CSCFCLayer: out = relu(x @ W + b).

Shapes: x [4096, 4096] f32, W [4096, 4096] f32, b [4096] f32 -> out [4096, 4096] f32.

Sharding: 2D over 8 cores -- batch split 4 ways x units split 2 ways. Each core
computes a [1024, 2048] slice of the output. Per-core DMA: xT slice 16 MiB +
W slice 32 MiB + out 8 MiB = 56 MiB (vs 80 MiB for 1D sharding), which puts the
kernel in the compute-bound regime (PE floor ~209 us at the measured 204 ns per
fp32r matmul).

x is pre-transposed on the host so the contraction dim lands on SBUF partitions
with no on-device transposes. The output is produced transposed (units on
partitions) so bias is a per-partition scalar and a single DVE tensor_scalar
(psum + bias, then max 0) fuses bias+relu while draining PSUM. Matmuls run in
float32r (full-rate fp32 PE path: 204 ns per [128x128x512] matmul measured).

Per core:
  xT_c [K=4096, M=1024] resident in SBUF (16 MiB)
  W_c  [K=4096, N=2048] streamed once (32 MiB) in [128, 8, 512] blocks
  outT_c [N=2048, M=1024]
  4 groups of (4 unit-chunks x 2 m-halves = 8 PSUM banks):
    k-accumulate 32 tiles; innermost over unit-chunks so the moving operand
    stays fixed for 4 consecutive matmuls; DVE bias+relu drain; DMA out.
"""

import os

import numpy as np

import concourse.bass as bass
import concourse.tile as tile
from concourse import bacc, mybir
from concourse.bass_utils import run_bass_kernel_spmd

N_CORES = 8
P_SHARD = 4  # batch split
Q_SHARD = 2  # units split
B = 4096
K = 4096
N = 4096
BS = B // P_SHARD  # 1024 batch rows per core
NS = N // Q_SHARD  # 2048 units per core
P = 128
KT = K // P  # 32 k-tiles
NCHUNK = NS // P  # 16 unit-chunks of 128 per core
NG = NS // 512  # 4 groups of 4 unit-chunks
MH = BS // 512  # 2 moving halves of the batch

MM_DT = {
    "f32r": mybir.dt.float32r,
    "f32": mybir.dt.float32,
    "bf16": mybir.dt.bfloat16,
}[os.environ.get("CSCFC_MM_DT", "f32r")]

# natural-out mapping: lhsT = xT chunks (stationary), rhs = W tiles (moving,
# fixed for 8 consecutive matmuls), psum in natural [batch, units] layout
NAT = os.environ.get("CSCFC_NAT", "0") == "1"


def _emit_loads(nc, xpool, bpool, xt, bt):
    # Resident x.T: [128 (k-partition), 32 (k-tile), 1024 (batch)]
    xt_r = xt.rearrange("(kt p) m -> p kt m", p=P)
    xt_sb = xpool.tile([P, KT, BS], xt.dtype, tag="xt_sb", name="xt_sb")
    n_pre = min(16, KT)  # split preload so the first matmuls start early
    for c in range(n_pre):
        sl = slice(c * (KT // n_pre), (c + 1) * (KT // n_pre))
        nc.sync.dma_start(xt_sb[:, sl, :], xt_r[:, sl, :])

    bias_sb = bpool.tile(
        [P, bt.shape[1]], mybir.dt.float32, tag="bias_sb", name="bias_sb"
    )
    nc.sync.dma_start(bias_sb[:], bt[:, :])
    return xt_sb, bias_sb


def _emit(nc, wpool, opool, psum_pool, xt_sb, bias_sb, w, outT):
    # groups of CPG unit-chunks x MH m-halves of PSUM banks.  j-innermost so
    # the MOVING operand (xt slice) stays fixed for CPG consecutive matmuls --
    # changing the moving AP per-mm costs ~70 ns/mm on HW.  The epilogue is a
    # single DVE tensor_scalar (psum + bias per-partition, then max 0) -- the
    # ScalarE ACTIVATE drain is ~2.5 us and serializes against PE PSUM writes.
    CPG = int(os.environ.get("CSCFC_CPG", "2"))  # unit-chunks per group
    # MSPLIT: each m-half is its own psum group (halves banks-in-flight at the
    # cost of streaming W twice)
    MSPLIT = os.environ.get("CSCFC_MSPLIT", "0") == "1"
    KTW = min(8, KT)  # k-tiles per W fetch block
    w_r = w.rearrange("(kb p) n -> p kb n", p=P)
    mh_groups = [[mh] for mh in range(MH)] if MSPLIT else [list(range(MH))]
    for g in range(NCHUNK // CPG):
        for mhs in mh_groups:
            psums = {
                (mh, j): psum_pool.tile(
                    [P, 512], mybir.dt.float32, tag="ps", name=f"ps_{g}_{mh}_{j}"
                )
                for mh in mhs
                for j in range(CPG)
            }
            for kb in range(KT // KTW):
                wt = wpool.tile(
                    [P, KTW, CPG * P], w.dtype, tag="wt", name=f"wt_{g}_{mhs[0]}_{kb}"
                )
                nc.sync.dma_start(
                    wt[:],
                    w_r[:, kb * KTW : (kb + 1) * KTW, g * CPG * P : (g + 1) * CPG * P],
                )
                for ki in range(KTW):
                    k = kb * KTW + ki
                    for mh in mhs:
                        for j in range(CPG):
                            nc.tensor.matmul(
                                psums[(mh, j)][:],
                                wt[:, ki, j * P : (j + 1) * P],
                                xt_sb[:, k, mh * 512 : (mh + 1) * 512],
                                start=(k == 0),
                                stop=(k == KT - 1),
                            )
            for mh in mhs:
                for j in range(CPG):
                    ch = g * CPG + j
                    ot = opool.tile(
                        [P, 512], mybir.dt.float32, tag="ot", name=f"ot_{ch}_{mh}"
                    )
                    nc.vector.tensor_scalar(
                        ot[:],
                        psums[(mh, j)][:],
                        bias_sb[:, ch : ch + 1],
                        0.0,
                        mybir.AluOpType.add,
                        mybir.AluOpType.max,
                    )
                    nc.sync.dma_start(
                        outT[ch * P : (ch + 1) * P, mh * 512 : (mh + 1) * 512], ot[:]
                    )


def _emit_nat(nc, wpool, opool, psum_pool, xt_sb, bias_sb, w, o):
    # rhs = W tile, fixed across the 8 m-chunk matmuls (moving-AP change costs
    # ~70 ns/mm; runs of 8 amortize it to ~9).  All 8 PSUM banks hold one
    # n-tile's m-chunks; DVE 2-op epilogue (add broadcast bias row, relu).
    KTW = min(8, KT)
    MC = BS // P  # 8 m-chunks
    w_r = w.rearrange("(kb p) n -> p kb n", p=P)
    for nt in range(NS // 512):
        psums = [
            psum_pool.tile([P, 512], mybir.dt.float32, tag="ps", name=f"ps_{nt}_{mc}")
            for mc in range(MC)
        ]
        for kb in range(KT // KTW):
            wt = wpool.tile([P, KTW, 512], w.dtype, tag="wt", name=f"wt_{nt}_{kb}")
            nc.sync.dma_start(
                wt[:], w_r[:, kb * KTW : (kb + 1) * KTW, nt * 512 : (nt + 1) * 512]
            )
            for ki in range(KTW):
                k = kb * KTW + ki
                for mc in range(MC):
                    nc.tensor.matmul(
                        psums[mc][:],
                        xt_sb[:, k, mc * P : (mc + 1) * P],
                        wt[:, ki, :],
                        start=(k == 0),
                        stop=(k == KT - 1),
                    )
        for mc in range(MC):
            ot = opool.tile([P, 512], mybir.dt.float32, tag="ot", name=f"ot_{nt}_{mc}")
            nc.vector.tensor_tensor(
                ot[:],
                psums[mc][:],
                bias_sb[:, nt * 512 : (nt + 1) * 512],
                mybir.AluOpType.add,
            )
            nc.vector.tensor_scalar_max(ot[:], ot[:], 0.0)
            nc.sync.dma_start(
                o[mc * P : (mc + 1) * P, nt * 512 : (nt + 1) * 512], ot[:]
            )


def _body(tc, xt, w, bt, outT, reps=1):
    nc = tc.nc
    with (
        tc.tile_pool(name="xpool", bufs=1) as xpool,
        tc.tile_pool(name="wpool", bufs=2) as wpool,
        tc.tile_pool(name="bpool", bufs=1) as bpool,
        tc.tile_pool(name="opool", bufs=8) as opool,
        tc.tile_pool(name="psum", bufs=8, space="PSUM") as psum_pool,
    ):
        xt_sb, bias_sb = _emit_loads(nc, xpool, bpool, xt, bt)
        emit = _emit_nat if NAT else _emit
        if reps > 1:
            # steady-state timing mode: resident loads stay outside the loop
            with tc.For_i(0, reps, 1):
                emit(nc, wpool, opool, psum_pool, xt_sb, bias_sb, w, outT)
        else:
            emit(nc, wpool, opool, psum_pool, xt_sb, bias_sb, w, outT)


def build_nc(mm_dt=MM_DT, reps=1, full_reps=1):
    nc = bacc.Bacc("TRN2", target_bir_lowering=False, debug=False)
    xt = nc.dram_tensor("xt", (K, BS), mm_dt, kind="ExternalInput")
    w = nc.dram_tensor("w", (K, NS), mm_dt, kind="ExternalInput")
    bt = nc.dram_tensor(
        "bt", (P, NS if NAT else NCHUNK), mybir.dt.float32, kind="ExternalInput"
    )
    outT = nc.dram_tensor(
        "outT", (BS, NS) if NAT else (NS, BS), mybir.dt.float32, kind="ExternalOutput"
    )
    with tile.TileContext(nc) as tc:
        if full_reps > 1:
            # timing variant: repeat the ENTIRE kernel (incl. resident loads)
            # so a reps-differential bounds the single-shot time from above
            nc2 = tc.nc
            with (
                tc.tile_pool(name="xpool", bufs=1) as xpool,
                tc.tile_pool(name="wpool", bufs=2) as wpool,
                tc.tile_pool(name="bpool", bufs=1) as bpool,
                tc.tile_pool(name="opool", bufs=8) as opool,
                tc.tile_pool(name="psum", bufs=8, space="PSUM") as psum_pool,
            ):
                emit = _emit_nat if NAT else _emit
                with tc.For_i(0, full_reps, 1):
                    xt_sb, bias_sb = _emit_loads(nc2, xpool, bpool, xt.ap(), bt.ap())
                    emit(nc2, wpool, opool, psum_pool, xt_sb, bias_sb, w.ap(), outT.ap())
        else:
            _body(tc, xt.ap(), w.ap(), bt.ap(), outT.ap(), reps=reps)
    nc.compile()
    return nc


_CACHED_NC = None


def _get_nc():
    global _CACHED_NC
    if _CACHED_NC is None:
        _CACHED_NC = build_nc()
    return _CACHED_NC


def make_in_maps(x, w, bias):
    x = np.asarray(x)
    w = np.asarray(w)
    bias = np.asarray(bias)
    np_dt = mybir.dt.np(MM_DT)
    xT = np.ascontiguousarray(x.T.astype(np_dt, copy=False))  # [K, B]
    wc = w.astype(np_dt, copy=False)
    bias = bias.astype(np.float32, copy=False)
    in_maps = []
    for c in range(N_CORES):
        pi, qi = divmod(c, Q_SHARD)
        in_maps.append(
            {
                "xt": np.ascontiguousarray(xT[:, pi * BS : (pi + 1) * BS]),
                "w": np.ascontiguousarray(wc[:, qi * NS : (qi + 1) * NS]),
                "bt": np.ascontiguousarray(
                    np.broadcast_to(bias[qi * NS : (qi + 1) * NS], (P, NS))
                )
                if NAT
                else np.ascontiguousarray(
                    bias[qi * NS : (qi + 1) * NS].reshape(NCHUNK, P).T
                ),
            }
        )
    return in_maps


def gather_out(results):
    out = np.empty((B, N), dtype=np.float32)
    for c in range(N_CORES):
        pi, qi = divmod(c, Q_SHARD)
        r = results[c]["outT"]
        out[pi * BS : (pi + 1) * BS, qi * NS : (qi + 1) * NS] = r if NAT else r.T
    return out


def _run(nc, x, w, bias, **spmd_kwargs):
    in_maps = make_in_maps(x, w, bias)
    res = run_bass_kernel_spmd(nc, in_maps, list(range(N_CORES)), **spmd_kwargs)
    return gather_out(res.results), res


def kernel(x, kernel, bias):
    try:
        out, _ = _run(_get_nc(), x, kernel, bias)
    except Exception:
        # transient device errors (e.g. NRT_EXEC_UNIT_UNRECOVERABLE) recover
        # on re-execution
        out, _ = _run(_get_nc(), x, kernel, bias)
    return out
